# revision 2
# baseline (speedup 1.0000x reference)
"""Trainium2 Bass kernel: dual-stream EMA scatter-mean memory update (fp8).

Problem: for two streams (rgb, ir), compute per-class means of 65536 feature
rows [2048] scattered by label into 1000 classes, then EMA-update the
[1000, 2048] memory banks where classes are present.

Strategy (class-sharded, no collectives, fp8 staging):
  - Host packs the 1000 classes into 8 bins (<=128 classes each, greedy LPT
    on per-class counts, independently per stream) so per-core row loads are
    near-balanced; routes each sample row to the core owning its class and
    rebases labels to bin-local slots.
  - Features are cast to fp8_e4m3 on the host (pure dtype staging; the 2e-2
    rel-err budget dwarfs the ~0.5% quantization noise after per-class
    averaging) and shipped partition-major ([128, chunks*2048]) so each DMA
    descriptor is a long contiguous run.
  - On device, per pair of 128-row chunks: DVE builds a [128, 2, 128] fp8
    one-hot against an iota row, TensorE runs DoubleRow fp8 matmuls
    (one-hot^T @ feats, two chunks per instruction) accumulating per-class
    sums in PSUM [128, 2048] fp32.
  - Counts are label-derived, so scale = sigma/count and coef = 1-sigma
    (present) come precomputed from the host; epilogue blends
    out = scale*psum + coef*mem (mem in bf16) and DMAs bf16 out.
  - Host scatters per-core rows of present classes back into the fp32
    memory banks.
"""
import math
from contextlib import ExitStack

import numpy as np
import ml_dtypes

import concourse.bass as bass
import concourse.tile as tile
from concourse import bacc, mybir
from concourse.bass_utils import run_bass_kernel_spmd

N = 65536
D = 2048
C = 1000
SIGMA = 0.2
N_CORES = 8
P = 128
SLOTS = 128      # class slots per core
NDT = D // 512   # psum d-tiles

FP8_NP = ml_dtypes.float8_e4m3

_NC_CACHE: dict = {}


def _build_nc(chunks: int, reps: int = 1, *, gn: int = 6, fbufs: int = 5,
              double_row: bool = True):
    assert gn % 2 == 0
    nc = bacc.Bacc("TRN2", target_bir_lowering=False, debug=False,
                   num_devices=N_CORES)
    f8 = mybir.dt.float8e4
    f32 = mybir.dt.float32
    bf16 = mybir.dt.bfloat16
    f_ap = [
        nc.dram_tensor(f"f{s}", [P, chunks * D], f8,
                       kind="ExternalInput").ap()
        for s in range(2)
    ]
    # columns: [0, chunks) = per-chunk local labels, chunks = scale,
    # chunks+1 = coef
    lab_ap = [
        nc.dram_tensor(f"lab{s}", [P, chunks + 2], f32,
                       kind="ExternalInput").ap()
        for s in range(2)
    ]
    mem_ap = [
        nc.dram_tensor(f"m{s}", [SLOTS, D], bf16, kind="ExternalInput").ap()
        for s in range(2)
    ]
    out_ap = nc.dram_tensor("out", [2, SLOTS, D], bf16,
                            kind="ExternalOutput").ap()

    with tile.TileContext(nc) as tc:
        with ExitStack() as ctx:
            const_pool = ctx.enter_context(tc.tile_pool(name="const", bufs=1))
            lpool = ctx.enter_context(tc.tile_pool(name="labs", bufs=2))
            fpool = ctx.enter_context(tc.tile_pool(name="feat", bufs=fbufs))
            ohpool = ctx.enter_context(tc.tile_pool(name="oh", bufs=6))
            mpool = ctx.enter_context(tc.tile_pool(name="mem", bufs=2))
            epool = ctx.enter_context(tc.tile_pool(name="ema", bufs=2))
            opool = ctx.enter_context(tc.tile_pool(name="obuf", bufs=2))
            ppool = ctx.enter_context(tc.tile_pool(name="psum", bufs=2,
                                                   space="PSUM"))

            iota_t = const_pool.tile([P, P], f32)
            nc.gpsimd.iota(iota_t[:, :], [[1, P]], channel_multiplier=0,
                           allow_small_or_imprecise_dtypes=True)

            def stream_body(s):
                labs = lpool.tile([P, chunks + 2], f32, tag="labs")
                nc.sync.dma_start(out=labs[:, :], in_=lab_ap[s][:, :])
                mem_t = mpool.tile([P, D], bf16, tag="mem")
                nc.scalar.dma_start(out=mem_t[:, :], in_=mem_ap[s][:, :])
                psum = ppool.tile([P, D], f32, tag="sums")

                views = []  # chunk k -> (3d tile view, local idx)
                for g0 in range(0, chunks, gn):
                    g = min(gn, chunks - g0)
                    ft = fpool.tile([P, g * D], f8, tag="fraw")
                    nc.sync.dma_start(out=ft[:, :],
                                      in_=f_ap[s][:, g0 * D:(g0 + g) * D])
                    ft3 = ft[:, :].rearrange("p (c d) -> p c d", c=g)
                    for c in range(g):
                        views.append((ft3, c))

                scale = labs[:, chunks:chunks + 1]
                coef = labs[:, chunks + 1:chunks + 2]

                for pi in range(chunks // 2):
                    k = 2 * pi
                    ft3, c = views[k]
                    ft3b, cb = views[k + 1]
                    assert ft3 is ft3b and cb == c + 1
                    first = k == 0
                    last = k + 2 == chunks
                    if double_row:
                        oh = ohpool.tile([P, 2 * P], f8, tag="oh")
                        oh3 = oh[:, :].rearrange("p (two c) -> p two c",
                                                 two=2)
                        for i in range(2):
                            nc.vector.tensor_scalar(
                                out=oh3[:, i, :], in0=iota_t[:, :],
                                scalar1=labs[:, k + i:k + i + 1],
                                scalar2=None,
                                op0=mybir.AluOpType.is_equal)
                        for j in range(NDT):
                            nc.tensor.matmul(
                                out=psum[:, 512 * j:512 * (j + 1)],
                                lhsT=oh3[:, :, :],
                                rhs=ft3[:, c:c + 2, 512 * j:512 * (j + 1)],
                                start=first, stop=last,
                                perf_mode=mybir.MatmulPerfMode.DoubleRow,
                                skip_group_check=True)
                    else:
                        for i in range(2):
                            oh = ohpool.tile([P, 2 * P], f8, tag="oh")
                            nc.vector.tensor_scalar(
                                out=oh[:, :P], in0=iota_t[:, :],
                                scalar1=labs[:, k + i:k + i + 1],
                                scalar2=None,
                                op0=mybir.AluOpType.is_equal)
                            for j in range(NDT):
                                nc.tensor.matmul(
                                    out=psum[:, 512 * j:512 * (j + 1)],
                                    lhsT=oh[:, :P],
                                    rhs=ft3[:, c + i,
                                            512 * j:512 * (j + 1)],
                                    start=first and i == 0,
                                    stop=last and i == 1,
                                    skip_group_check=True)
                if chunks % 2:
                    k = chunks - 1
                    ft3, c = views[k]
                    oh = ohpool.tile([P, 2 * P], f8, tag="oh")
                    nc.vector.tensor_scalar(
                        out=oh[:, :P], in0=iota_t[:, :],
                        scalar1=labs[:, k:k + 1], scalar2=None,
                        op0=mybir.AluOpType.is_equal)
                    for j in range(NDT):
                        nc.tensor.matmul(
                            out=psum[:, 512 * j:512 * (j + 1)],
                            lhsT=oh[:, :P],
                            rhs=ft3[:, c, 512 * j:512 * (j + 1)],
                            start=(chunks == 1), stop=True,
                            skip_group_check=True)

                obuf = opool.tile([P, D], bf16, tag="obuf")
                for j in range(NDT):
                    sl = slice(512 * j, 512 * (j + 1))
                    t1 = epool.tile([P, 512], f32, tag="t1")
                    nc.scalar.mul(t1[:, :], psum[:, sl], scale)
                    t2 = epool.tile([P, 512], f32, tag="t2")
                    nc.vector.tensor_scalar_mul(t2[:, :], mem_t[:, sl], coef)
                    nc.vector.tensor_tensor(
                        out=obuf[:, sl], in0=t1[:, :], in1=t2[:, :],
                        op=mybir.AluOpType.add)
                nc.scalar.dma_start(out=out_ap[s, :, :], in_=obuf[:, :])

            for _rep in range(reps):
                for s in range(2):
                    stream_body(s)

    nc.compile()
    return nc


_TUNED = dict(gn=6, fbufs=5, double_row=True)


def _get_nc(chunks: int, reps: int = 1):
    key = (chunks, reps)
    if key not in _NC_CACHE:
        _NC_CACHE[key] = _build_nc(chunks, reps, **_TUNED)
    return _NC_CACHE[key]


def _pack_classes(counts: np.ndarray):
    """Greedy LPT: pack 1000 class counts into 8 bins of <=SLOTS classes,
    minimizing the max bin row-load. Returns (assign[C], slot[C], loads[8],
    class_lists per bin)."""
    order = np.argsort(-counts, kind="stable")
    loads = np.zeros(N_CORES, np.int64)
    nclasses = np.zeros(N_CORES, np.int64)
    assign = np.empty(C, np.int32)
    slot = np.empty(C, np.int32)
    class_lists = [[] for _ in range(N_CORES)]
    for cls in order:
        elig = np.flatnonzero(nclasses < SLOTS)
        b = elig[np.argmin(loads[elig])]
        assign[cls] = b
        slot[cls] = nclasses[b]
        class_lists[b].append(cls)
        nclasses[b] += 1
        loads[b] += counts[cls]
    return assign, slot, loads, [np.asarray(cl) for cl in class_lists]


def _stage(inputs: dict):
    """Host staging: pack classes, route+quantize rows, build per-core maps.

    Returns (in_maps, chunks); also stashes unshard info on the in_maps list
    via attribute _meta for kernel() to assemble the output.
    """
    feats = [np.asarray(inputs["rgb_feats"], dtype=np.float32),
             np.asarray(inputs["ir_feats"], dtype=np.float32)]
    mems = [np.asarray(inputs["vis_memory"], dtype=np.float32),
            np.asarray(inputs["ir_memory"], dtype=np.float32)]
    labels = [np.asarray(inputs["rgb_labels"]).astype(np.int64),
              np.asarray(inputs["ir_labels"]).astype(np.int64)]

    packs = []
    max_rows = 1
    for s in range(2):
        counts = np.bincount(labels[s], minlength=C)
        assign, slot, loads, class_lists = _pack_classes(counts)
        packs.append((counts, assign, slot, class_lists))
        max_rows = max(max_rows, int(loads.max()))
    chunks = math.ceil(max_rows / P)
    pad_rows = chunks * P

    in_maps = [dict() for _ in range(N_CORES)]
    meta = []  # per stream: class_lists, counts
    for s in range(2):
        counts, assign, slot, class_lists = packs[s]
        f8 = feats[s].astype(FP8_NP)
        core_of = assign[labels[s]]
        order = np.argsort(core_of, kind="stable")
        bounds = np.searchsorted(core_of[order], np.arange(N_CORES + 1))
        local_lab_all = slot[labels[s]]
        for m in range(N_CORES):
            lo, hi = int(bounds[m]), int(bounds[m + 1])
            n_m = hi - lo
            assert n_m <= pad_rows
            rows = order[lo:hi]
            fl = np.zeros((pad_rows, D), FP8_NP)
            fl[:n_m] = f8[rows]
            # partition-major: [128, chunks*D], partition p holds chunks c
            # at columns [c*D, (c+1)*D) = row c*128+p of the routed order
            flayout = np.ascontiguousarray(
                fl.reshape(chunks, P, D).transpose(1, 0, 2).reshape(P, -1))
            ll = np.full((pad_rows,), -1.0, np.float32)
            ll[:n_m] = local_lab_all[rows].astype(np.float32)
            labs2d = np.empty((P, chunks + 2), np.float32)
            labs2d[:, :chunks] = ll.reshape(chunks, P).T
            cls = class_lists[m]
            cl_counts = np.zeros(SLOTS, np.float32)
            cl_counts[:len(cls)] = counts[cls]
            present = cl_counts > 0
            labs2d[:, chunks] = np.where(present,
                                         SIGMA / np.maximum(cl_counts, 1.0),
                                         0.0)
            labs2d[:, chunks + 1] = np.where(present, 1.0 - SIGMA, 1.0)
            mem_m = np.zeros((SLOTS, D), ml_dtypes.bfloat16)
            mem_m[:len(cls)] = mems[s][cls].astype(ml_dtypes.bfloat16)
            in_maps[m][f"f{s}"] = flayout
            in_maps[m][f"lab{s}"] = labs2d
            in_maps[m][f"m{s}"] = mem_m
        meta.append((class_lists, counts))
    return in_maps, chunks, meta


def _unshard(results, meta, mems):
    out = np.stack([m.astype(np.float32) for m in mems], axis=0)
    for s in range(2):
        class_lists, counts = meta[s]
        for m in range(N_CORES):
            cls = class_lists[m]
            dev = np.asarray(results[m]["out"][s][:len(cls)]).astype(
                np.float32)
            pres = counts[cls] > 0
            out[s][cls[pres]] = dev[pres]
    return out


def _run(inputs: dict, trace: bool = False, trace_cores=None, tmpdir=None):
    in_maps, chunks, meta = _stage(inputs)
    nc = _get_nc(chunks)
    try:
        res = run_bass_kernel_spmd(
            nc, in_maps, core_ids=list(range(N_CORES)), trace=trace,
            trace_cores=trace_cores, tmpdir=tmpdir)
    except ModuleNotFoundError:
        import os
        os.environ["BASS_NEVER_TRACE"] = "1"
        res = run_bass_kernel_spmd(
            nc, in_maps, core_ids=list(range(N_CORES)), trace=False,
            tmpdir=tmpdir)
    mems = [np.asarray(inputs["vis_memory"], dtype=np.float32),
            np.asarray(inputs["ir_memory"], dtype=np.float32)]
    out = _unshard(res.results, meta, mems)
    return out, res


def kernel(**inputs) -> np.ndarray:
    out, _ = _run(inputs, trace=False)
    return out
